# revision 14
# baseline (speedup 1.0000x reference)
"""Trainium2 Bass kernel for BailingMoeV2 sparse MoE block (8-core expert-parallel).

Contract: kernel(**inputs) takes FULL numpy inputs, returns FULL [T, H] f32 output.
Internally: shards across 8 NeuronCores (4 routed experts per core + 1/8 of the
shared expert each), runs one SPMD Bass/Tile kernel, host-sums the partial outputs.

Device algorithm (per core):
  - router logits in split-precision bf16 (x = hi + lo; three bf16 matmuls
    reproduce f32 logits to ~4e-6), sigmoid on ACT
  - group-limited top-k routing entirely on-chip (vector.max top-8, match_replace)
  - dense expert compute in bf16, combine weights folded into h before down-proj;
    down-proj accumulates all 4 local experts + shared expert in PSUM
Expert groups are permuted per-core (swap group 0 <-> group c) so that each core's
4 local experts always occupy expert columns 0..3; group-limited top-k is
invariant under group permutation.
"""
import sys

sys.path.insert(0, "/opt/trn_rl_repo")

import numpy as np
import ml_dtypes

import concourse.bass as bass
from concourse import bacc
import concourse.mybir as mybir
import concourse.tile as tile
from concourse.bass_utils import run_bass_kernel_spmd
from concourse.masks import make_identity
from contextlib import ExitStack

BF16 = ml_dtypes.bfloat16

T, H, I, E, G = 1024, 2048, 1024, 32, 8
TOPK_GROUP, TOP_K = 4, 8
SCALE = 2.5
EPG = E // G            # 4 experts per group
NCORES = 8
ELOC = E // NCORES      # 4 local experts per core (== one group)
IS = I // NCORES        # 128 shared-expert intermediate per core
KT = H // 128           # 16 k-tiles over hidden
MT = T // 128           # 8 token tiles
ITL = I // 128          # 8 i-tiles over moe intermediate
NH = H // 512           # 4 output column chunks
BIGNEG = 1.0e4

F32 = mybir.dt.float32
BF = mybir.dt.bfloat16


def build_nc():
    nc = bacc.Bacc()
    xTb = nc.declare_dram_parameter("xTb", [128, KT, T], BF, isOutput=False)
    xTl = nc.declare_dram_parameter("xTl", [128, KT, T], BF, isOutput=False)
    gwh = nc.declare_dram_parameter("gwh", [128, KT, E], BF, isOutput=False)
    gwl = nc.declare_dram_parameter("gwl", [128, KT, E], BF, isOutput=False)
    biasb = nc.declare_dram_parameter("biasb", [128, MT * E], F32, isOutput=False)
    wgu = nc.declare_dram_parameter("wgu", [ELOC, 16, 128, KT, 128], BF, isOutput=False)
    wd = nc.declare_dram_parameter("wd", [ELOC, NH, 128, ITL, 512], BF, isOutput=False)
    sgu = nc.declare_dram_parameter("sgu", [128, 2, KT, 128], BF, isOutput=False)
    sd = nc.declare_dram_parameter("sd", [128, NH, 512], BF, isOutput=False)
    eye4d = nc.declare_dram_parameter("eye4", [4, ELOC * 128], BF, isOutput=False)
    out = nc.declare_dram_parameter("out", [T, H], F32, isOutput=True)

    with tile.TileContext(nc) as tc:
        with ExitStack() as ctx:
            cst = ctx.enter_context(tc.tile_pool(name="cst", bufs=1))
            rt = ctx.enter_context(tc.tile_pool(name="rt", bufs=1))
            ps_g = ctx.enter_context(tc.tile_pool(name="ps_g", bufs=4, space="PSUM"))
            ps_d = ctx.enter_context(tc.tile_pool(name="ps_d", bufs=2, space="PSUM"))

            # ---- constants (resident) ----
            xTb_sb = cst.tile([128, KT, T], BF)
            nc.sync.dma_start(out=xTb_sb[:], in_=xTb[:])
            biasb_sb = cst.tile([128, MT * E], F32)
            nc.sync.dma_start(out=biasb_sb[:], in_=biasb[:])
            ident = cst.tile([128, 128], F32)
            make_identity(nc, ident[:])
            eye4 = cst.tile([4, ELOC * 128], BF)
            nc.sync.dma_start(out=eye4[:], in_=eye4d[:])
            sgu_sb = cst.tile([128, 2, KT, 128], BF)
            nc.sync.dma_start(out=sgu_sb[:], in_=sgu[:])
            sd_sb = cst.tile([128, NH, 512], BF)
            nc.sync.dma_start(out=sd_sb[:], in_=sd[:])

            # ---- router matmul: split-precision bf16 ----
            with tc.tile_pool(name="xlo", bufs=1) as xlo:
                xTl_sb = xlo.tile([128, KT, T], BF)
                nc.sync.dma_start(out=xTl_sb[:], in_=xTl[:])
                gwh_sb = cst.tile([128, KT, E], BF)
                nc.sync.dma_start(out=gwh_sb[:], in_=gwh[:])
                gwl_sb = cst.tile([128, KT, E], BF)
                nc.sync.dma_start(out=gwl_sb[:], in_=gwl[:])

                logits_ps = ps_g.tile([128, MT * E], F32, tag="g")
                for ti in range(MT):
                    terms = [(xTb_sb, gwh_sb), (xTb_sb, gwl_sb), (xTl_sb, gwh_sb)]
                    for p, (xs, gw) in enumerate(terms):
                        for k in range(KT):
                            nc.tensor.matmul(
                                logits_ps[:, ti * E:(ti + 1) * E],
                                lhsT=xs[:, k, ti * 128:(ti + 1) * 128],
                                rhs=gw[:, k, :],
                                start=(p == 0 and k == 0),
                                stop=(p == 2 and k == KT - 1),
                            )

                # ---- routing (token-major [128, MT, E]) ----
                scores = rt.tile([128, MT, E], F32)
                nc.scalar.activation(
                    scores[:].rearrange("p m e -> p (m e)"), logits_ps[:],
                    mybir.ActivationFunctionType.Sigmoid,
                )
            srt = rt.tile([128, MT, E], F32)
            nc.vector.tensor_tensor(
                out=srt[:].rearrange("p m e -> p (m e)"),
                in0=scores[:].rearrange("p m e -> p (m e)"),
                in1=biasb_sb[:], op=mybir.AluOpType.add,
            )
            # group scores: max of pairwise sums within each group of 4
            gs = rt.tile([128, MT, G], F32)
            tmp_g = rt.tile([128, MT, G], F32)
            pairs = [(0, 1), (0, 2), (0, 3), (1, 2), (1, 3), (2, 3)]
            sv = srt[:].rearrange("p m (g e) -> p m g e", e=EPG)
            for idx, (i, j) in enumerate(pairs):
                dst = gs if idx == 0 else tmp_g
                nc.vector.tensor_tensor(
                    out=dst[:], in0=sv[:, :, :, i], in1=sv[:, :, :, j],
                    op=mybir.AluOpType.add,
                )
                if idx > 0:
                    nc.vector.tensor_tensor(
                        out=gs[:], in0=gs[:], in1=tmp_g[:], op=mybir.AluOpType.max,
                    )
            # group mask: keep groups >= 4th largest group score
            g8 = rt.tile([128, 8], F32)
            gmask = rt.tile([128, MT, G], F32)
            for ti in range(MT):
                nc.vector.max(out=g8[:], in_=gs[:, ti, :])
                nc.vector.tensor_scalar(
                    out=gmask[:, ti, :], in0=gs[:, ti, :],
                    scalar1=g8[:, TOPK_GROUP - 1:TOPK_GROUP], scalar2=None,
                    op0=mybir.AluOpType.is_ge,
                )
            # expand group mask to experts; exact masking
            emask = rt.tile([128, MT, E], F32)
            ev = emask[:].rearrange("p m (g e) -> p m g e", e=EPG)
            for i in range(EPG):
                nc.vector.tensor_copy(out=ev[:, :, :, i], in_=gmask[:])
            ms = rt.tile([128, MT, E], F32)
            nc.vector.tensor_tensor(
                out=ms[:], in0=srt[:], in1=emask[:], op=mybir.AluOpType.mult,
            )
            negc = rt.tile([128, MT, E], F32)
            nc.vector.tensor_scalar(
                out=negc[:], in0=emask[:], scalar1=1.0, scalar2=BIGNEG,
                op0=mybir.AluOpType.subtract, op1=mybir.AluOpType.mult,
            )
            nc.vector.tensor_tensor(
                out=ms[:], in0=ms[:], in1=negc[:], op=mybir.AluOpType.add,
            )
            # top-8 selection mask
            v8 = rt.tile([128, MT * 8], F32)
            zap = rt.tile([128, MT, E], F32)
            for ti in range(MT):
                nc.vector.max(out=v8[:, ti * 8:(ti + 1) * 8], in_=ms[:, ti, :])
                nc.vector.match_replace(
                    out=zap[:, ti, :], in_to_replace=v8[:, ti * 8:(ti + 1) * 8],
                    in_values=ms[:, ti, :], imm_value=-3.0 * BIGNEG,
                )
            sel = rt.tile([128, MT, E], F32)
            nc.vector.tensor_tensor(
                out=sel[:], in0=ms[:], in1=zap[:], op=mybir.AluOpType.is_gt,
            )
            wdense = rt.tile([128, MT, E], F32)
            nc.vector.tensor_tensor(
                out=wdense[:], in0=scores[:], in1=sel[:], op=mybir.AluOpType.mult,
            )
            den = rt.tile([128, MT], F32)
            nc.vector.reduce_sum(
                out=den[:], in_=wdense[:], axis=mybir.AxisListType.X,
            )
            nc.vector.tensor_scalar(
                out=den[:], in0=den[:], scalar1=1.0 / SCALE, scalar2=None,
                op0=mybir.AluOpType.mult,
            )
            rden = rt.tile([128, MT], F32)
            nc.vector.reciprocal(out=rden[:], in_=den[:])
            combine = rt.tile([128, MT, E], F32)
            for ti in range(MT):
                nc.vector.tensor_scalar(
                    out=combine[:, ti, :], in0=wdense[:, ti, :],
                    scalar1=rden[:, ti:ti + 1], scalar2=None,
                    op0=mybir.AluOpType.mult,
                )
            # combT [E, T] (bf16) via PE transpose of each token tile
            combTb = rt.tile([E, MT, 128], BF)
            for ti in range(MT):
                ct_ps = ps_d.tile([128, 512], F32, tag="d")
                nc.tensor.transpose(
                    out=ct_ps[:E, :128], in_=combine[:, ti, :], identity=ident[:],
                )
                nc.vector.tensor_copy(out=combTb[:, ti, :], in_=ct_ps[:E, :128])
            combT_flat = combTb[:].rearrange("e m p -> e (m p)")
            # Wb[le] = combine row le broadcast across 128 partitions (bf16)
            wb = rt.tile([128, ELOC, T], BF)
            for le in range(ELOC):
                for nch in range(2):
                    wb_ps = ps_g.tile([128, 512], F32, tag="g")
                    nc.tensor.matmul(
                        wb_ps[:],
                        lhsT=eye4[:, le * 128:(le + 1) * 128],
                        rhs=combT_flat[0:ELOC, nch * 512:(nch + 1) * 512],
                        start=True, stop=True,
                    )
                    nc.vector.tensor_copy(
                        out=wb[:, le, nch * 512:(nch + 1) * 512], in_=wb_ps[:],
                    )

            # ---- shared expert: h_s.T [IS=128, T] ----
            hsT = cst.tile([128, T], BF)
            for nch in range(2):
                psg = ps_g.tile([128, 512], F32, tag="g")
                psu = ps_g.tile([128, 512], F32, tag="g")
                for k in range(KT):
                    nc.tensor.matmul(
                        psg[:], lhsT=sgu_sb[:, 0, k, :],
                        rhs=xTb_sb[:, k, nch * 512:(nch + 1) * 512],
                        start=(k == 0), stop=(k == KT - 1),
                    )
                for k in range(KT):
                    nc.tensor.matmul(
                        psu[:], lhsT=sgu_sb[:, 1, k, :],
                        rhs=xTb_sb[:, k, nch * 512:(nch + 1) * 512],
                        start=(k == 0), stop=(k == KT - 1),
                    )
                sg = rt.tile([128, 512], BF, tag="sg")
                nc.scalar.activation(
                    sg[:], psg[:], mybir.ActivationFunctionType.Silu,
                )
                nc.vector.tensor_tensor(
                    out=hsT[:, nch * 512:(nch + 1) * 512], in0=sg[:], in1=psu[:],
                    op=mybir.AluOpType.mult,
                )

            # ---- routed experts: gate_up for all 4 (hT stays resident) ----
            with ExitStack() as ctx2:
                ht_p = ctx2.enter_context(tc.tile_pool(name="ht_p", bufs=1))
                hts = []
                for le in range(ELOC):
                    hT_le = ht_p.tile([128, ITL, T], BF, tag=f"ht{le}", name=f"ht{le}")
                    hts.append(hT_le)
                with tc.tile_pool(name="wgu_p", bufs=4) as wgu_p:
                    for le in range(ELOC):
                        hT = hts[le]
                        for m in range(ITL):
                            wg_sb = wgu_p.tile([128, KT, 128], BF, tag="wg")
                            nc.sync.dma_start(out=wg_sb[:], in_=wgu[le, m])
                            wu_sb = wgu_p.tile([128, KT, 128], BF, tag="wg")
                            nc.sync.dma_start(out=wu_sb[:], in_=wgu[le, ITL + m])
                            for nch in range(2):
                                psg = ps_g.tile([128, 512], F32, tag="g")
                                psu = ps_g.tile([128, 512], F32, tag="g")
                                for k in range(KT):
                                    nc.tensor.matmul(
                                        psg[:], lhsT=wg_sb[:, k, :],
                                        rhs=xTb_sb[:, k, nch * 512:(nch + 1) * 512],
                                        start=(k == 0), stop=(k == KT - 1),
                                    )
                                for k in range(KT):
                                    nc.tensor.matmul(
                                        psu[:], lhsT=wu_sb[:, k, :],
                                        rhs=xTb_sb[:, k, nch * 512:(nch + 1) * 512],
                                        start=(k == 0), stop=(k == KT - 1),
                                    )
                                sg = rt.tile([128, 512], BF, tag="sg")
                                nc.scalar.activation(
                                    sg[:], psg[:],
                                    mybir.ActivationFunctionType.Silu,
                                )
                                hu = rt.tile([128, 512], BF, tag="hu")
                                nc.vector.tensor_tensor(
                                    out=hu[:], in0=sg[:], in1=psu[:],
                                    op=mybir.AluOpType.mult,
                                )
                                nc.vector.tensor_tensor(
                                    out=hT[:, m, nch * 512:(nch + 1) * 512],
                                    in0=hu[:],
                                    in1=wb[:, le, nch * 512:(nch + 1) * 512],
                                    op=mybir.AluOpType.mult,
                                )

                # ---- down-proj: accumulate 4 experts + shared in PSUM ----
                with tc.tile_pool(name="wd_p", bufs=5) as wd_p:
                    for nh in range(NH):
                        wd_sbs = []
                        for le in range(ELOC):
                            wd_sb = wd_p.tile([128, ITL, 512], BF, tag="wd")
                            nc.sync.dma_start(out=wd_sb[:], in_=wd[le, nh])
                            wd_sbs.append(wd_sb)
                        for mt in range(MT):
                            ps = ps_d.tile([128, 512], F32, tag="d")
                            nc.tensor.matmul(
                                ps[:], lhsT=hsT[:, mt * 128:(mt + 1) * 128],
                                rhs=sd_sb[:, nh, :], start=True, stop=False,
                            )
                            for le in range(ELOC):
                                for k in range(ITL):
                                    nc.tensor.matmul(
                                        ps[:],
                                        lhsT=hts[le][:, k, mt * 128:(mt + 1) * 128],
                                        rhs=wd_sbs[le][:, k, :],
                                        start=False,
                                        stop=(le == ELOC - 1 and k == ITL - 1),
                                    )
                            ysb = rt.tile([128, 512], F32, tag="ysb")
                            nc.scalar.activation(
                                ysb[:], ps[:], mybir.ActivationFunctionType.Copy,
                            )
                            nc.sync.dma_start(
                                out=out[mt * 128:(mt + 1) * 128,
                                        nh * 512:(nh + 1) * 512],
                                in_=ysb[:],
                            )
    nc.compile()
    return nc


def _tile_xT(x):
    # [T, H] -> [128, KT, T]  with xT[p, k, t] = x[t, k*128+p]
    return np.ascontiguousarray(x.T.reshape(KT, 128, T).transpose(1, 0, 2))


def _prep_in_maps(hidden_states, gate_w, expert_bias, w_gate_up, w_down,
                  shared_gate_up, shared_down):
    x = np.asarray(hidden_states, np.float32)
    gate_w = np.asarray(gate_w, np.float32)
    expert_bias = np.asarray(expert_bias, np.float32)
    w_gate_up = np.asarray(w_gate_up)
    w_down = np.asarray(w_down)
    shared_gate_up = np.asarray(shared_gate_up)
    shared_down = np.asarray(shared_down)

    xT = _tile_xT(x)
    xTb = xT.astype(BF16)
    xTl = (xT - xTb.astype(np.float32)).astype(BF16)

    in_maps = []
    for c in range(NCORES):
        pg = list(range(G))
        pg[0], pg[c] = pg[c], pg[0]
        perm = np.concatenate([np.arange(g * EPG, (g + 1) * EPG) for g in pg])
        gw_p = gate_w[perm]
        bias_p = expert_bias[perm]

        gwT = np.ascontiguousarray(gw_p.T.reshape(KT, 128, E).transpose(1, 0, 2))
        gwh = gwT.astype(BF16)
        gwl = (gwT - gwh.astype(np.float32)).astype(BF16)
        biasb = np.broadcast_to(np.tile(bias_p, MT)[None, :], (128, MT * E)).copy()

        # local experts: global ids 4c..4c+3
        wgu_t = np.empty((ELOC, 16, 128, KT, 128), BF16)
        wd_t = np.empty((ELOC, NH, 128, ITL, 512), BF16)
        for le in range(ELOC):
            e = EPG * c + le
            wT = w_gate_up[e].T.astype(np.float32)          # [H, 2I]
            wgu_t[le] = wT.reshape(KT, 128, 16, 128).transpose(2, 1, 0, 3).astype(BF16)
            dT = w_down[e].T.astype(np.float32)             # [I, H]
            wd_t[le] = dT.reshape(ITL, 128, NH, 512).transpose(2, 1, 0, 3).astype(BF16)

        sguT = shared_gate_up.T.astype(np.float32)          # [H, 2I]
        sgu_t = np.empty((128, 2, KT, 128), BF16)
        sgu_t[:, 0] = sguT[:, c * IS:(c + 1) * IS].reshape(KT, 128, 128).transpose(1, 0, 2).astype(BF16)
        sgu_t[:, 1] = sguT[:, I + c * IS:I + (c + 1) * IS].reshape(KT, 128, 128).transpose(1, 0, 2).astype(BF16)
        sdT = shared_down.T.astype(np.float32)              # [I, H]
        sd_t = sdT[c * IS:(c + 1) * IS].reshape(128, NH, 512).astype(BF16)

        eye4_h = np.zeros((4, ELOC * 128), BF16)
        for le in range(ELOC):
            eye4_h[le, le * 128:(le + 1) * 128] = 1.0
        in_maps.append({
            "xTb": xTb, "xTl": xTl, "gwh": gwh, "gwl": gwl, "biasb": biasb,
            "wgu": wgu_t, "wd": wd_t, "sgu": sgu_t, "sd": sd_t, "eye4": eye4_h,
        })
    return in_maps


_NC_CACHE = {}


def run(inputs, trace=False):
    if "nc" not in _NC_CACHE:
        _NC_CACHE["nc"] = build_nc()
    nc = _NC_CACHE["nc"]
    in_maps = _prep_in_maps(**inputs)
    res = run_bass_kernel_spmd(nc, in_maps, core_ids=list(range(NCORES)),
                               trace=trace)
    y = np.zeros((T, H), np.float64)
    for c in range(NCORES):
        y += res.results[c]["out"].astype(np.float64)
    return y.astype(np.float32), res


def kernel(**inputs):
    y, _ = run(inputs, trace=False)
    return y


# revision 15
# speedup vs baseline: 1.0099x; 1.0099x over previous
"""Trainium2 Bass kernel for BailingMoeV2 sparse MoE block (8-core expert-parallel).

Contract: kernel(**inputs) takes FULL numpy inputs, returns FULL [T, H] f32 output.
Internally: shards across 8 NeuronCores (4 routed experts per core + 1/8 of the
shared expert each), runs one SPMD Bass/Tile kernel, host-sums the partial outputs.

Device algorithm (per core):
  - router logits in split-precision bf16 (x = hi + lo; three bf16 matmuls
    reproduce f32 logits to ~4e-6), sigmoid on ACT
  - group-limited top-k routing entirely on-chip (vector.max top-8, match_replace)
  - dense expert compute in bf16, combine weights folded into h before down-proj;
    down-proj accumulates all 4 local experts + shared expert in PSUM
Expert groups are permuted per-core (swap group 0 <-> group c) so that each core's
4 local experts always occupy expert columns 0..3; group-limited top-k is
invariant under group permutation.
"""
import sys

sys.path.insert(0, "/opt/trn_rl_repo")

import numpy as np
import ml_dtypes

import concourse.bass as bass
from concourse import bacc
import concourse.mybir as mybir
import concourse.tile as tile
from concourse.bass_utils import run_bass_kernel_spmd
from concourse.masks import make_identity
from contextlib import ExitStack

BF16 = ml_dtypes.bfloat16

T, H, I, E, G = 1024, 2048, 1024, 32, 8
TOPK_GROUP, TOP_K = 4, 8
SCALE = 2.5
EPG = E // G            # 4 experts per group
NCORES = 8
ELOC = E // NCORES      # 4 local experts per core (== one group)
IS = I // NCORES        # 128 shared-expert intermediate per core
KT = H // 128           # 16 k-tiles over hidden
MT = T // 128           # 8 token tiles
ITL = I // 128          # 8 i-tiles over moe intermediate
NH = H // 512           # 4 output column chunks
BIGNEG = 1.0e4

F32 = mybir.dt.float32
BF = mybir.dt.bfloat16


def build_nc():
    nc = bacc.Bacc()
    xTb = nc.declare_dram_parameter("xTb", [128, KT, T], BF, isOutput=False)
    xTl = nc.declare_dram_parameter("xTl", [128, KT, T], BF, isOutput=False)
    gwh = nc.declare_dram_parameter("gwh", [128, KT, E], BF, isOutput=False)
    gwl = nc.declare_dram_parameter("gwl", [128, KT, E], BF, isOutput=False)
    biasb = nc.declare_dram_parameter("biasb", [128, MT * E], F32, isOutput=False)
    wgu = nc.declare_dram_parameter("wgu", [ELOC, 16, 128, KT, 128], BF, isOutput=False)
    wd = nc.declare_dram_parameter("wd", [ELOC, NH, 128, ITL, 512], BF, isOutput=False)
    sgu = nc.declare_dram_parameter("sgu", [128, 2, KT, 128], BF, isOutput=False)
    sd = nc.declare_dram_parameter("sd", [128, NH, 512], BF, isOutput=False)
    eye4d = nc.declare_dram_parameter("eye4", [4, ELOC * 128], BF, isOutput=False)
    out = nc.declare_dram_parameter("out", [T, H], F32, isOutput=True)

    with tile.TileContext(nc) as tc:
        with ExitStack() as ctx:
            cst = ctx.enter_context(tc.tile_pool(name="cst", bufs=1))
            rt = ctx.enter_context(tc.tile_pool(name="rt", bufs=1))
            ps_g = ctx.enter_context(tc.tile_pool(name="ps_g", bufs=6, space="PSUM"))
            ps_d = ctx.enter_context(tc.tile_pool(name="ps_d", bufs=2, space="PSUM"))

            # ---- constants (resident) ----
            xTb_sb = cst.tile([128, KT, T], BF)
            nc.sync.dma_start(out=xTb_sb[:], in_=xTb[:])
            biasb_sb = cst.tile([128, MT * E], F32)
            nc.sync.dma_start(out=biasb_sb[:], in_=biasb[:])
            ident = cst.tile([128, 128], F32)
            make_identity(nc, ident[:])
            eye4 = cst.tile([4, ELOC * 128], BF)
            nc.sync.dma_start(out=eye4[:], in_=eye4d[:])
            sgu_sb = cst.tile([128, 2, KT, 128], BF)
            nc.sync.dma_start(out=sgu_sb[:], in_=sgu[:])
            sd_sb = cst.tile([128, NH, 512], BF)
            nc.sync.dma_start(out=sd_sb[:], in_=sd[:])

            # ---- router matmul: split-precision bf16 ----
            with tc.tile_pool(name="xlo", bufs=1) as xlo:
                xTl_sb = xlo.tile([128, KT, T], BF)
                nc.sync.dma_start(out=xTl_sb[:], in_=xTl[:])
                gwh_sb = cst.tile([128, KT, E], BF)
                nc.sync.dma_start(out=gwh_sb[:], in_=gwh[:])
                gwl_sb = cst.tile([128, KT, E], BF)
                nc.sync.dma_start(out=gwl_sb[:], in_=gwl[:])

                logits_ps = ps_g.tile([128, MT * E], F32, tag="g")
                for ti in range(MT):
                    terms = [(xTb_sb, gwh_sb), (xTb_sb, gwl_sb), (xTl_sb, gwh_sb)]
                    for p, (xs, gw) in enumerate(terms):
                        for k in range(KT):
                            nc.tensor.matmul(
                                logits_ps[:, ti * E:(ti + 1) * E],
                                lhsT=xs[:, k, ti * 128:(ti + 1) * 128],
                                rhs=gw[:, k, :],
                                start=(p == 0 and k == 0),
                                stop=(p == 2 and k == KT - 1),
                            )

                # ---- routing (token-major [128, MT, E]) ----
                scores = rt.tile([128, MT, E], F32)
                nc.scalar.activation(
                    scores[:].rearrange("p m e -> p (m e)"), logits_ps[:],
                    mybir.ActivationFunctionType.Sigmoid,
                )
            srt = rt.tile([128, MT, E], F32)
            nc.vector.tensor_tensor(
                out=srt[:].rearrange("p m e -> p (m e)"),
                in0=scores[:].rearrange("p m e -> p (m e)"),
                in1=biasb_sb[:], op=mybir.AluOpType.add,
            )
            # group scores: max of pairwise sums within each group of 4
            gs = rt.tile([128, MT, G], F32)
            tmp_g = rt.tile([128, MT, G], F32)
            pairs = [(0, 1), (0, 2), (0, 3), (1, 2), (1, 3), (2, 3)]
            sv = srt[:].rearrange("p m (g e) -> p m g e", e=EPG)
            for idx, (i, j) in enumerate(pairs):
                dst = gs if idx == 0 else tmp_g
                nc.vector.tensor_tensor(
                    out=dst[:], in0=sv[:, :, :, i], in1=sv[:, :, :, j],
                    op=mybir.AluOpType.add,
                )
                if idx > 0:
                    nc.vector.tensor_tensor(
                        out=gs[:], in0=gs[:], in1=tmp_g[:], op=mybir.AluOpType.max,
                    )
            # group mask: keep groups >= 4th largest group score
            g8 = rt.tile([128, 8], F32)
            gmask = rt.tile([128, MT, G], F32)
            for ti in range(MT):
                nc.vector.max(out=g8[:], in_=gs[:, ti, :])
                nc.vector.tensor_scalar(
                    out=gmask[:, ti, :], in0=gs[:, ti, :],
                    scalar1=g8[:, TOPK_GROUP - 1:TOPK_GROUP], scalar2=None,
                    op0=mybir.AluOpType.is_ge,
                )
            # expand group mask to experts; exact masking
            emask = rt.tile([128, MT, E], F32)
            ev = emask[:].rearrange("p m (g e) -> p m g e", e=EPG)
            for i in range(EPG):
                nc.vector.tensor_copy(out=ev[:, :, :, i], in_=gmask[:])
            ms = rt.tile([128, MT, E], F32)
            nc.vector.tensor_tensor(
                out=ms[:], in0=srt[:], in1=emask[:], op=mybir.AluOpType.mult,
            )
            negc = rt.tile([128, MT, E], F32)
            nc.vector.tensor_scalar(
                out=negc[:], in0=emask[:], scalar1=1.0, scalar2=BIGNEG,
                op0=mybir.AluOpType.subtract, op1=mybir.AluOpType.mult,
            )
            nc.vector.tensor_tensor(
                out=ms[:], in0=ms[:], in1=negc[:], op=mybir.AluOpType.add,
            )
            # top-8 selection mask
            v8 = rt.tile([128, MT * 8], F32)
            zap = rt.tile([128, MT, E], F32)
            for ti in range(MT):
                nc.vector.max(out=v8[:, ti * 8:(ti + 1) * 8], in_=ms[:, ti, :])
                nc.vector.match_replace(
                    out=zap[:, ti, :], in_to_replace=v8[:, ti * 8:(ti + 1) * 8],
                    in_values=ms[:, ti, :], imm_value=-3.0 * BIGNEG,
                )
            sel = rt.tile([128, MT, E], F32)
            nc.vector.tensor_tensor(
                out=sel[:], in0=ms[:], in1=zap[:], op=mybir.AluOpType.is_gt,
            )
            wdense = rt.tile([128, MT, E], F32)
            nc.vector.tensor_tensor(
                out=wdense[:], in0=scores[:], in1=sel[:], op=mybir.AluOpType.mult,
            )
            den = rt.tile([128, MT], F32)
            nc.vector.reduce_sum(
                out=den[:], in_=wdense[:], axis=mybir.AxisListType.X,
            )
            nc.vector.tensor_scalar(
                out=den[:], in0=den[:], scalar1=1.0 / SCALE, scalar2=None,
                op0=mybir.AluOpType.mult,
            )
            rden = rt.tile([128, MT], F32)
            nc.vector.reciprocal(out=rden[:], in_=den[:])
            combine = rt.tile([128, MT, E], F32)
            for ti in range(MT):
                nc.vector.tensor_scalar(
                    out=combine[:, ti, :], in0=wdense[:, ti, :],
                    scalar1=rden[:, ti:ti + 1], scalar2=None,
                    op0=mybir.AluOpType.mult,
                )
            # combT [E, T] (bf16) via PE transpose of each token tile
            combTb = rt.tile([E, MT, 128], BF)
            for ti in range(MT):
                ct_ps = ps_d.tile([128, 512], F32, tag="d")
                nc.tensor.transpose(
                    out=ct_ps[:E, :128], in_=combine[:, ti, :], identity=ident[:],
                )
                nc.vector.tensor_copy(out=combTb[:, ti, :], in_=ct_ps[:E, :128])
            combT_flat = combTb[:].rearrange("e m p -> e (m p)")
            # Wb[le] = combine row le broadcast across 128 partitions (bf16)
            wb = rt.tile([128, ELOC, T], BF)
            for le in range(ELOC):
                for nch in range(2):
                    wb_ps = ps_g.tile([128, 512], F32, tag="g")
                    nc.tensor.matmul(
                        wb_ps[:],
                        lhsT=eye4[:, le * 128:(le + 1) * 128],
                        rhs=combT_flat[0:ELOC, nch * 512:(nch + 1) * 512],
                        start=True, stop=True,
                    )
                    nc.vector.tensor_copy(
                        out=wb[:, le, nch * 512:(nch + 1) * 512], in_=wb_ps[:],
                    )

            # ---- shared expert: h_s.T [IS=128, T] ----
            hsT = cst.tile([128, T], BF)
            for nch in range(2):
                psg = ps_g.tile([128, 512], F32, tag="g")
                psu = ps_g.tile([128, 512], F32, tag="g")
                for k in range(KT):
                    nc.tensor.matmul(
                        psg[:], lhsT=sgu_sb[:, 0, k, :],
                        rhs=xTb_sb[:, k, nch * 512:(nch + 1) * 512],
                        start=(k == 0), stop=(k == KT - 1),
                    )
                for k in range(KT):
                    nc.tensor.matmul(
                        psu[:], lhsT=sgu_sb[:, 1, k, :],
                        rhs=xTb_sb[:, k, nch * 512:(nch + 1) * 512],
                        start=(k == 0), stop=(k == KT - 1),
                    )
                sg = rt.tile([128, 512], BF, tag="sg")
                nc.scalar.activation(
                    sg[:], psg[:], mybir.ActivationFunctionType.Silu,
                )
                nc.vector.tensor_tensor(
                    out=hsT[:, nch * 512:(nch + 1) * 512], in0=sg[:], in1=psu[:],
                    op=mybir.AluOpType.mult,
                )

            # ---- routed experts: gate_up for all 4 (hT stays resident) ----
            with ExitStack() as ctx2:
                ht_p = ctx2.enter_context(tc.tile_pool(name="ht_p", bufs=1))
                hts = []
                for le in range(ELOC):
                    hT_le = ht_p.tile([128, ITL, T], BF, tag=f"ht{le}", name=f"ht{le}")
                    hts.append(hT_le)
                with tc.tile_pool(name="wgu_p", bufs=4) as wgu_p:
                    for le in range(ELOC):
                        hT = hts[le]
                        for m in range(ITL):
                            wg_sb = wgu_p.tile([128, KT, 128], BF, tag="wg")
                            nc.sync.dma_start(out=wg_sb[:], in_=wgu[le, m])
                            wu_sb = wgu_p.tile([128, KT, 128], BF, tag="wg")
                            nc.sync.dma_start(out=wu_sb[:], in_=wgu[le, ITL + m])
                            for nch in range(2):
                                psg = ps_g.tile([128, 512], F32, tag="g")
                                psu = ps_g.tile([128, 512], F32, tag="g")
                                for k in range(KT):
                                    nc.tensor.matmul(
                                        psg[:], lhsT=wg_sb[:, k, :],
                                        rhs=xTb_sb[:, k, nch * 512:(nch + 1) * 512],
                                        start=(k == 0), stop=(k == KT - 1),
                                    )
                                for k in range(KT):
                                    nc.tensor.matmul(
                                        psu[:], lhsT=wu_sb[:, k, :],
                                        rhs=xTb_sb[:, k, nch * 512:(nch + 1) * 512],
                                        start=(k == 0), stop=(k == KT - 1),
                                    )
                                sg = rt.tile([128, 512], BF, tag="sg")
                                nc.scalar.activation(
                                    sg[:], psg[:],
                                    mybir.ActivationFunctionType.Silu,
                                )
                                hu = rt.tile([128, 512], BF, tag="hu")
                                nc.vector.tensor_tensor(
                                    out=hu[:], in0=sg[:], in1=psu[:],
                                    op=mybir.AluOpType.mult,
                                )
                                nc.vector.tensor_tensor(
                                    out=hT[:, m, nch * 512:(nch + 1) * 512],
                                    in0=hu[:],
                                    in1=wb[:, le, nch * 512:(nch + 1) * 512],
                                    op=mybir.AluOpType.mult,
                                )

                # ---- down-proj: accumulate 4 experts + shared in PSUM ----
                with tc.tile_pool(name="wd_p", bufs=5) as wd_p:
                    for nh in range(NH):
                        wd_sbs = []
                        for le in range(ELOC):
                            wd_sb = wd_p.tile([128, ITL, 512], BF, tag="wd")
                            nc.sync.dma_start(out=wd_sb[:], in_=wd[le, nh])
                            wd_sbs.append(wd_sb)
                        for mt in range(MT):
                            ps = ps_d.tile([128, 512], F32, tag="d")
                            nc.tensor.matmul(
                                ps[:], lhsT=hsT[:, mt * 128:(mt + 1) * 128],
                                rhs=sd_sb[:, nh, :], start=True, stop=False,
                            )
                            for le in range(ELOC):
                                for k in range(ITL):
                                    nc.tensor.matmul(
                                        ps[:],
                                        lhsT=hts[le][:, k, mt * 128:(mt + 1) * 128],
                                        rhs=wd_sbs[le][:, k, :],
                                        start=False,
                                        stop=(le == ELOC - 1 and k == ITL - 1),
                                    )
                            ysb = rt.tile([128, 512], F32, tag="ysb")
                            nc.scalar.activation(
                                ysb[:], ps[:], mybir.ActivationFunctionType.Copy,
                            )
                            nc.sync.dma_start(
                                out=out[mt * 128:(mt + 1) * 128,
                                        nh * 512:(nh + 1) * 512],
                                in_=ysb[:],
                            )
    nc.compile()
    return nc


def _tile_xT(x):
    # [T, H] -> [128, KT, T]  with xT[p, k, t] = x[t, k*128+p]
    return np.ascontiguousarray(x.T.reshape(KT, 128, T).transpose(1, 0, 2))


def _prep_in_maps(hidden_states, gate_w, expert_bias, w_gate_up, w_down,
                  shared_gate_up, shared_down):
    x = np.asarray(hidden_states, np.float32)
    gate_w = np.asarray(gate_w, np.float32)
    expert_bias = np.asarray(expert_bias, np.float32)
    w_gate_up = np.asarray(w_gate_up)
    w_down = np.asarray(w_down)
    shared_gate_up = np.asarray(shared_gate_up)
    shared_down = np.asarray(shared_down)

    xT = _tile_xT(x)
    xTb = xT.astype(BF16)
    xTl = (xT - xTb.astype(np.float32)).astype(BF16)

    in_maps = []
    for c in range(NCORES):
        pg = list(range(G))
        pg[0], pg[c] = pg[c], pg[0]
        perm = np.concatenate([np.arange(g * EPG, (g + 1) * EPG) for g in pg])
        gw_p = gate_w[perm]
        bias_p = expert_bias[perm]

        gwT = np.ascontiguousarray(gw_p.T.reshape(KT, 128, E).transpose(1, 0, 2))
        gwh = gwT.astype(BF16)
        gwl = (gwT - gwh.astype(np.float32)).astype(BF16)
        biasb = np.broadcast_to(np.tile(bias_p, MT)[None, :], (128, MT * E)).copy()

        # local experts: global ids 4c..4c+3
        wgu_t = np.empty((ELOC, 16, 128, KT, 128), BF16)
        wd_t = np.empty((ELOC, NH, 128, ITL, 512), BF16)
        for le in range(ELOC):
            e = EPG * c + le
            wT = w_gate_up[e].T.astype(np.float32)          # [H, 2I]
            wgu_t[le] = wT.reshape(KT, 128, 16, 128).transpose(2, 1, 0, 3).astype(BF16)
            dT = w_down[e].T.astype(np.float32)             # [I, H]
            wd_t[le] = dT.reshape(ITL, 128, NH, 512).transpose(2, 1, 0, 3).astype(BF16)

        sguT = shared_gate_up.T.astype(np.float32)          # [H, 2I]
        sgu_t = np.empty((128, 2, KT, 128), BF16)
        sgu_t[:, 0] = sguT[:, c * IS:(c + 1) * IS].reshape(KT, 128, 128).transpose(1, 0, 2).astype(BF16)
        sgu_t[:, 1] = sguT[:, I + c * IS:I + (c + 1) * IS].reshape(KT, 128, 128).transpose(1, 0, 2).astype(BF16)
        sdT = shared_down.T.astype(np.float32)              # [I, H]
        sd_t = sdT[c * IS:(c + 1) * IS].reshape(128, NH, 512).astype(BF16)

        eye4_h = np.zeros((4, ELOC * 128), BF16)
        for le in range(ELOC):
            eye4_h[le, le * 128:(le + 1) * 128] = 1.0
        in_maps.append({
            "xTb": xTb, "xTl": xTl, "gwh": gwh, "gwl": gwl, "biasb": biasb,
            "wgu": wgu_t, "wd": wd_t, "sgu": sgu_t, "sd": sd_t, "eye4": eye4_h,
        })
    return in_maps


_NC_CACHE = {}


def run(inputs, trace=False):
    if "nc" not in _NC_CACHE:
        _NC_CACHE["nc"] = build_nc()
    nc = _NC_CACHE["nc"]
    in_maps = _prep_in_maps(**inputs)
    res = run_bass_kernel_spmd(nc, in_maps, core_ids=list(range(NCORES)),
                               trace=trace)
    y = np.zeros((T, H), np.float64)
    for c in range(NCORES):
        y += res.results[c]["out"].astype(np.float64)
    return y.astype(np.float32), res


def kernel(**inputs):
    y, _ = run(inputs, trace=False)
    return y


# revision 16
# speedup vs baseline: 1.0173x; 1.0074x over previous
"""Trainium2 Bass kernel for BailingMoeV2 sparse MoE block (8-core expert-parallel).

Contract: kernel(**inputs) takes FULL numpy inputs, returns FULL [T, H] f32 output.
Internally: shards across 8 NeuronCores (4 routed experts per core + 1/8 of the
shared expert each), runs one SPMD Bass/Tile kernel, host-sums the partial outputs.

Device algorithm (per core):
  - router logits in split-precision bf16 (x = hi + lo; three bf16 matmuls
    reproduce f32 logits to ~4e-6), sigmoid on ACT
  - group-limited top-k routing entirely on-chip (vector.max top-8, match_replace)
  - dense expert compute in bf16, combine weights folded into h before down-proj;
    down-proj accumulates all 4 local experts + shared expert in PSUM
Expert groups are permuted per-core (swap group 0 <-> group c) so that each core's
4 local experts always occupy expert columns 0..3; group-limited top-k is
invariant under group permutation.
"""
import sys

sys.path.insert(0, "/opt/trn_rl_repo")

import numpy as np
import ml_dtypes

import concourse.bass as bass
from concourse import bacc
import concourse.mybir as mybir
import concourse.tile as tile
from concourse.bass_utils import run_bass_kernel_spmd
from concourse.masks import make_identity
from contextlib import ExitStack

BF16 = ml_dtypes.bfloat16

T, H, I, E, G = 1024, 2048, 1024, 32, 8
TOPK_GROUP, TOP_K = 4, 8
SCALE = 2.5
EPG = E // G            # 4 experts per group
NCORES = 8
ELOC = E // NCORES      # 4 local experts per core (== one group)
IS = I // NCORES        # 128 shared-expert intermediate per core
KT = H // 128           # 16 k-tiles over hidden
MT = T // 128           # 8 token tiles
ITL = I // 128          # 8 i-tiles over moe intermediate
NH = H // 512           # 4 output column chunks
BIGNEG = 1.0e4

F32 = mybir.dt.float32
BF = mybir.dt.bfloat16


def build_nc():
    nc = bacc.Bacc()
    xTb = nc.declare_dram_parameter("xTb", [128, KT, T], BF, isOutput=False)
    xTl = nc.declare_dram_parameter("xTl", [128, KT, T], BF, isOutput=False)
    gwh = nc.declare_dram_parameter("gwh", [128, KT, E], BF, isOutput=False)
    gwl = nc.declare_dram_parameter("gwl", [128, KT, E], BF, isOutput=False)
    biasb = nc.declare_dram_parameter("biasb", [128, MT * E], F32, isOutput=False)
    wgu = nc.declare_dram_parameter("wgu", [ELOC, 16, 128, KT, 128], BF, isOutput=False)
    wd = nc.declare_dram_parameter("wd", [ELOC, NH, 128, ITL, 512], BF, isOutput=False)
    sgu = nc.declare_dram_parameter("sgu", [128, 2, KT, 128], BF, isOutput=False)
    sd = nc.declare_dram_parameter("sd", [128, NH, 512], BF, isOutput=False)
    eye4d = nc.declare_dram_parameter("eye4", [4, ELOC * 128], BF, isOutput=False)
    out = nc.declare_dram_parameter("out", [T, H], F32, isOutput=True)

    with tile.TileContext(nc) as tc:
        with ExitStack() as ctx:
            cst = ctx.enter_context(tc.tile_pool(name="cst", bufs=1))
            rt = ctx.enter_context(tc.tile_pool(name="rt", bufs=1))
            ps_g = ctx.enter_context(tc.tile_pool(name="ps_g", bufs=6, space="PSUM"))
            ps_d = ctx.enter_context(tc.tile_pool(name="ps_d", bufs=2, space="PSUM"))

            # ---- constants (router inputs first; x split across DMA queues) ----
            gwh_sb = cst.tile([128, KT, E], BF)
            nc.sync.dma_start(out=gwh_sb[:], in_=gwh[:])
            gwl_sb = cst.tile([128, KT, E], BF)
            nc.sync.dma_start(out=gwl_sb[:], in_=gwl[:])
            xTb_sb = cst.tile([128, KT, T], BF)
            for kc in range(0, KT, 2):
                nc.sync.dma_start(out=xTb_sb[:, kc:kc + 2, :], in_=xTb[:, kc:kc + 2, :])
            sgu_sb = cst.tile([128, 2, KT, 128], BF)
            nc.sync.dma_start(out=sgu_sb[:], in_=sgu[:])
            biasb_sb = cst.tile([128, MT * E], F32)
            nc.sync.dma_start(out=biasb_sb[:], in_=biasb[:])
            ident = cst.tile([128, 128], F32)
            make_identity(nc, ident[:])
            eye4 = cst.tile([4, ELOC * 128], BF)
            nc.sync.dma_start(out=eye4[:], in_=eye4d[:])
            sd_sb = cst.tile([128, NH, 512], BF)
            nc.sync.dma_start(out=sd_sb[:], in_=sd[:])

            # ---- router matmul: split-precision bf16 ----
            with tc.tile_pool(name="xlo", bufs=1) as xlo:
                xTl_sb = xlo.tile([128, KT, T], BF)
                for kc in range(0, KT, 4):
                    nc.sync.dma_start(out=xTl_sb[:, kc:kc + 4, :], in_=xTl[:, kc:kc + 4, :])

                logits_ps = ps_g.tile([128, MT * E], F32, tag="g")
                for ti in range(MT):
                    terms = [(xTb_sb, gwh_sb), (xTb_sb, gwl_sb), (xTl_sb, gwh_sb)]
                    for p, (xs, gw) in enumerate(terms):
                        for k in range(KT):
                            nc.tensor.matmul(
                                logits_ps[:, ti * E:(ti + 1) * E],
                                lhsT=xs[:, k, ti * 128:(ti + 1) * 128],
                                rhs=gw[:, k, :],
                                start=(p == 0 and k == 0),
                                stop=(p == 2 and k == KT - 1),
                            )

                # ---- routing (token-major [128, MT, E]) ----
                scores = rt.tile([128, MT, E], F32)
                nc.scalar.activation(
                    scores[:].rearrange("p m e -> p (m e)"), logits_ps[:],
                    mybir.ActivationFunctionType.Sigmoid,
                )
            # ---- shared expert: h_s.T [IS=128, T] ----
            hsT = cst.tile([128, T], BF)
            for nch in range(2):
                psg = ps_g.tile([128, 512], F32, tag="g")
                psu = ps_g.tile([128, 512], F32, tag="g")
                for k in range(KT):
                    nc.tensor.matmul(
                        psg[:], lhsT=sgu_sb[:, 0, k, :],
                        rhs=xTb_sb[:, k, nch * 512:(nch + 1) * 512],
                        start=(k == 0), stop=(k == KT - 1),
                    )
                for k in range(KT):
                    nc.tensor.matmul(
                        psu[:], lhsT=sgu_sb[:, 1, k, :],
                        rhs=xTb_sb[:, k, nch * 512:(nch + 1) * 512],
                        start=(k == 0), stop=(k == KT - 1),
                    )
                sg = rt.tile([128, 512], BF, tag="sg")
                nc.scalar.activation(
                    sg[:], psg[:], mybir.ActivationFunctionType.Silu,
                )
                nc.vector.tensor_tensor(
                    out=hsT[:, nch * 512:(nch + 1) * 512], in0=sg[:], in1=psu[:],
                    op=mybir.AluOpType.mult,
                )

            srt = rt.tile([128, MT, E], F32)
            nc.vector.tensor_tensor(
                out=srt[:].rearrange("p m e -> p (m e)"),
                in0=scores[:].rearrange("p m e -> p (m e)"),
                in1=biasb_sb[:], op=mybir.AluOpType.add,
            )
            # group scores: max of pairwise sums within each group of 4
            gs = rt.tile([128, MT, G], F32)
            tmp_g = rt.tile([128, MT, G], F32)
            pairs = [(0, 1), (0, 2), (0, 3), (1, 2), (1, 3), (2, 3)]
            sv = srt[:].rearrange("p m (g e) -> p m g e", e=EPG)
            for idx, (i, j) in enumerate(pairs):
                dst = gs if idx == 0 else tmp_g
                nc.vector.tensor_tensor(
                    out=dst[:], in0=sv[:, :, :, i], in1=sv[:, :, :, j],
                    op=mybir.AluOpType.add,
                )
                if idx > 0:
                    nc.vector.tensor_tensor(
                        out=gs[:], in0=gs[:], in1=tmp_g[:], op=mybir.AluOpType.max,
                    )
            # group mask: keep groups >= 4th largest group score
            g8 = rt.tile([128, 8], F32)
            gmask = rt.tile([128, MT, G], F32)
            for ti in range(MT):
                nc.vector.max(out=g8[:], in_=gs[:, ti, :])
                nc.vector.tensor_scalar(
                    out=gmask[:, ti, :], in0=gs[:, ti, :],
                    scalar1=g8[:, TOPK_GROUP - 1:TOPK_GROUP], scalar2=None,
                    op0=mybir.AluOpType.is_ge,
                )
            # expand group mask to experts; exact masking
            emask = rt.tile([128, MT, E], F32)
            ev = emask[:].rearrange("p m (g e) -> p m g e", e=EPG)
            for i in range(EPG):
                nc.vector.tensor_copy(out=ev[:, :, :, i], in_=gmask[:])
            ms = rt.tile([128, MT, E], F32)
            nc.vector.tensor_tensor(
                out=ms[:], in0=srt[:], in1=emask[:], op=mybir.AluOpType.mult,
            )
            negc = rt.tile([128, MT, E], F32)
            nc.vector.tensor_scalar(
                out=negc[:], in0=emask[:], scalar1=1.0, scalar2=BIGNEG,
                op0=mybir.AluOpType.subtract, op1=mybir.AluOpType.mult,
            )
            nc.vector.tensor_tensor(
                out=ms[:], in0=ms[:], in1=negc[:], op=mybir.AluOpType.add,
            )
            # top-8 selection mask
            v8 = rt.tile([128, MT * 8], F32)
            zap = rt.tile([128, MT, E], F32)
            for ti in range(MT):
                nc.vector.max(out=v8[:, ti * 8:(ti + 1) * 8], in_=ms[:, ti, :])
                nc.vector.match_replace(
                    out=zap[:, ti, :], in_to_replace=v8[:, ti * 8:(ti + 1) * 8],
                    in_values=ms[:, ti, :], imm_value=-3.0 * BIGNEG,
                )
            sel = rt.tile([128, MT, E], F32)
            nc.vector.tensor_tensor(
                out=sel[:], in0=ms[:], in1=zap[:], op=mybir.AluOpType.is_gt,
            )
            wdense = rt.tile([128, MT, E], F32)
            nc.vector.tensor_tensor(
                out=wdense[:], in0=scores[:], in1=sel[:], op=mybir.AluOpType.mult,
            )
            den = rt.tile([128, MT], F32)
            nc.vector.reduce_sum(
                out=den[:], in_=wdense[:], axis=mybir.AxisListType.X,
            )
            nc.vector.tensor_scalar(
                out=den[:], in0=den[:], scalar1=1.0 / SCALE, scalar2=None,
                op0=mybir.AluOpType.mult,
            )
            rden = rt.tile([128, MT], F32)
            nc.vector.reciprocal(out=rden[:], in_=den[:])
            combine = rt.tile([128, MT, E], F32)
            for ti in range(MT):
                nc.vector.tensor_scalar(
                    out=combine[:, ti, :], in0=wdense[:, ti, :],
                    scalar1=rden[:, ti:ti + 1], scalar2=None,
                    op0=mybir.AluOpType.mult,
                )
            # combT [E, T] (bf16) via PE transpose of each token tile
            combTb = rt.tile([E, MT, 128], BF)
            for ti in range(MT):
                ct_ps = ps_d.tile([128, 512], F32, tag="d")
                nc.tensor.transpose(
                    out=ct_ps[:E, :128], in_=combine[:, ti, :], identity=ident[:],
                )
                nc.vector.tensor_copy(out=combTb[:, ti, :], in_=ct_ps[:E, :128])
            combT_flat = combTb[:].rearrange("e m p -> e (m p)")
            # Wb[le] = combine row le broadcast across 128 partitions (bf16)
            wb = rt.tile([128, ELOC, T], BF)
            for le in range(ELOC):
                for nch in range(2):
                    wb_ps = ps_g.tile([128, 512], F32, tag="g")
                    nc.tensor.matmul(
                        wb_ps[:],
                        lhsT=eye4[:, le * 128:(le + 1) * 128],
                        rhs=combT_flat[0:ELOC, nch * 512:(nch + 1) * 512],
                        start=True, stop=True,
                    )
                    nc.vector.tensor_copy(
                        out=wb[:, le, nch * 512:(nch + 1) * 512], in_=wb_ps[:],
                    )

            # ---- routed experts: gate_up for all 4 (hT stays resident) ----
            with ExitStack() as ctx2:
                ht_p = ctx2.enter_context(tc.tile_pool(name="ht_p", bufs=1))
                hts = []
                for le in range(ELOC):
                    hT_le = ht_p.tile([128, ITL, T], BF, tag=f"ht{le}", name=f"ht{le}")
                    hts.append(hT_le)
                with tc.tile_pool(name="wgu_p", bufs=4) as wgu_p:
                    for le in range(ELOC):
                        hT = hts[le]
                        for m in range(ITL):
                            wg_sb = wgu_p.tile([128, KT, 128], BF, tag="wg")
                            nc.sync.dma_start(out=wg_sb[:], in_=wgu[le, m])
                            wu_sb = wgu_p.tile([128, KT, 128], BF, tag="wg")
                            nc.sync.dma_start(out=wu_sb[:], in_=wgu[le, ITL + m])
                            for nch in range(2):
                                psg = ps_g.tile([128, 512], F32, tag="g")
                                psu = ps_g.tile([128, 512], F32, tag="g")
                                for k in range(KT):
                                    nc.tensor.matmul(
                                        psg[:], lhsT=wg_sb[:, k, :],
                                        rhs=xTb_sb[:, k, nch * 512:(nch + 1) * 512],
                                        start=(k == 0), stop=(k == KT - 1),
                                    )
                                for k in range(KT):
                                    nc.tensor.matmul(
                                        psu[:], lhsT=wu_sb[:, k, :],
                                        rhs=xTb_sb[:, k, nch * 512:(nch + 1) * 512],
                                        start=(k == 0), stop=(k == KT - 1),
                                    )
                                sg = rt.tile([128, 512], BF, tag="sg")
                                nc.scalar.activation(
                                    sg[:], psg[:],
                                    mybir.ActivationFunctionType.Silu,
                                )
                                hu = rt.tile([128, 512], BF, tag="hu")
                                nc.vector.tensor_tensor(
                                    out=hu[:], in0=sg[:], in1=psu[:],
                                    op=mybir.AluOpType.mult,
                                )
                                nc.vector.tensor_tensor(
                                    out=hT[:, m, nch * 512:(nch + 1) * 512],
                                    in0=hu[:],
                                    in1=wb[:, le, nch * 512:(nch + 1) * 512],
                                    op=mybir.AluOpType.mult,
                                )

                # ---- down-proj: accumulate 4 experts + shared in PSUM ----
                with tc.tile_pool(name="wd_p", bufs=5) as wd_p:
                    for nh in range(NH):
                        wd_sbs = []
                        for le in range(ELOC):
                            wd_sb = wd_p.tile([128, ITL, 512], BF, tag="wd")
                            nc.sync.dma_start(out=wd_sb[:], in_=wd[le, nh])
                            wd_sbs.append(wd_sb)
                        for mt in range(MT):
                            ps = ps_d.tile([128, 512], F32, tag="d")
                            nc.tensor.matmul(
                                ps[:], lhsT=hsT[:, mt * 128:(mt + 1) * 128],
                                rhs=sd_sb[:, nh, :], start=True, stop=False,
                            )
                            for le in range(ELOC):
                                for k in range(ITL):
                                    nc.tensor.matmul(
                                        ps[:],
                                        lhsT=hts[le][:, k, mt * 128:(mt + 1) * 128],
                                        rhs=wd_sbs[le][:, k, :],
                                        start=False,
                                        stop=(le == ELOC - 1 and k == ITL - 1),
                                    )
                            ysb = rt.tile([128, 512], F32, tag="ysb")
                            nc.scalar.activation(
                                ysb[:], ps[:], mybir.ActivationFunctionType.Copy,
                            )
                            nc.sync.dma_start(
                                out=out[mt * 128:(mt + 1) * 128,
                                        nh * 512:(nh + 1) * 512],
                                in_=ysb[:],
                            )
    nc.compile()
    return nc


def _tile_xT(x):
    # [T, H] -> [128, KT, T]  with xT[p, k, t] = x[t, k*128+p]
    return np.ascontiguousarray(x.T.reshape(KT, 128, T).transpose(1, 0, 2))


def _prep_in_maps(hidden_states, gate_w, expert_bias, w_gate_up, w_down,
                  shared_gate_up, shared_down):
    x = np.asarray(hidden_states, np.float32)
    gate_w = np.asarray(gate_w, np.float32)
    expert_bias = np.asarray(expert_bias, np.float32)
    w_gate_up = np.asarray(w_gate_up)
    w_down = np.asarray(w_down)
    shared_gate_up = np.asarray(shared_gate_up)
    shared_down = np.asarray(shared_down)

    xT = _tile_xT(x)
    xTb = xT.astype(BF16)
    xTl = (xT - xTb.astype(np.float32)).astype(BF16)

    in_maps = []
    for c in range(NCORES):
        pg = list(range(G))
        pg[0], pg[c] = pg[c], pg[0]
        perm = np.concatenate([np.arange(g * EPG, (g + 1) * EPG) for g in pg])
        gw_p = gate_w[perm]
        bias_p = expert_bias[perm]

        gwT = np.ascontiguousarray(gw_p.T.reshape(KT, 128, E).transpose(1, 0, 2))
        gwh = gwT.astype(BF16)
        gwl = (gwT - gwh.astype(np.float32)).astype(BF16)
        biasb = np.broadcast_to(np.tile(bias_p, MT)[None, :], (128, MT * E)).copy()

        # local experts: global ids 4c..4c+3
        wgu_t = np.empty((ELOC, 16, 128, KT, 128), BF16)
        wd_t = np.empty((ELOC, NH, 128, ITL, 512), BF16)
        for le in range(ELOC):
            e = EPG * c + le
            wT = w_gate_up[e].T.astype(np.float32)          # [H, 2I]
            wgu_t[le] = wT.reshape(KT, 128, 16, 128).transpose(2, 1, 0, 3).astype(BF16)
            dT = w_down[e].T.astype(np.float32)             # [I, H]
            wd_t[le] = dT.reshape(ITL, 128, NH, 512).transpose(2, 1, 0, 3).astype(BF16)

        sguT = shared_gate_up.T.astype(np.float32)          # [H, 2I]
        sgu_t = np.empty((128, 2, KT, 128), BF16)
        sgu_t[:, 0] = sguT[:, c * IS:(c + 1) * IS].reshape(KT, 128, 128).transpose(1, 0, 2).astype(BF16)
        sgu_t[:, 1] = sguT[:, I + c * IS:I + (c + 1) * IS].reshape(KT, 128, 128).transpose(1, 0, 2).astype(BF16)
        sdT = shared_down.T.astype(np.float32)              # [I, H]
        sd_t = sdT[c * IS:(c + 1) * IS].reshape(128, NH, 512).astype(BF16)

        eye4_h = np.zeros((4, ELOC * 128), BF16)
        for le in range(ELOC):
            eye4_h[le, le * 128:(le + 1) * 128] = 1.0
        in_maps.append({
            "xTb": xTb, "xTl": xTl, "gwh": gwh, "gwl": gwl, "biasb": biasb,
            "wgu": wgu_t, "wd": wd_t, "sgu": sgu_t, "sd": sd_t, "eye4": eye4_h,
        })
    return in_maps


_NC_CACHE = {}


def run(inputs, trace=False):
    if "nc" not in _NC_CACHE:
        _NC_CACHE["nc"] = build_nc()
    nc = _NC_CACHE["nc"]
    in_maps = _prep_in_maps(**inputs)
    res = run_bass_kernel_spmd(nc, in_maps, core_ids=list(range(NCORES)),
                               trace=trace)
    y = np.zeros((T, H), np.float64)
    for c in range(NCORES):
        y += res.results[c]["out"].astype(np.float64)
    return y.astype(np.float32), res


def kernel(**inputs):
    y, _ = run(inputs, trace=False)
    return y
